# revision 67
# baseline (speedup 1.0000x reference)
"""AttnBlock (GroupNorm -> QKV -> full attention -> proj + residual) on 8
Trainium2 NeuronCores, data-parallel over batch (b=8, one sample per core).

Transposeless fp8 design. Per core (sample):
  h_q = fp8(Sh*GroupNorm(x)); A = (wq.T wk)/sqrt(c), W = wp wv folded on
  host to fp8 (k / final proj never computed).
  u_q = fp8(Su*(A.T h + g)) via fp8 DoubleRow matmuls (K=256/pass).
  sT[j,i] = scores transposed, computed directly (lhsT = h j-cols), so no
  PE transposes and no PSUM->SBUF att copies are needed; exp written
  straight to fp8 attT slabs by ACT (scale/bias folded into the exp).
  Row sums of the quantized exp via a fp8 DoubleRow ones-matmul; the
  softmax normalization is applied per-column at the output stage using a
  PE-broadcast reciprocal row. ho = vp.T @ attT in fp8 DoubleRow; bias via
  a 1-row f32r matmul into the same PSUM accumulation; residual added on
  DVE from resident x tiles.
"""

import functools
import math

import numpy as np

B = 8
C = 512
W = 2048
G = 32
EPS = 1e-6
P = 128
CT = C // P          # 4 channel tiles
NW = W // 512        # 4 w-chunks of 512
IT = W // P          # 16 j-tiles
NPR = 2              # contraction pairs (2 x 256 = 512)

# fp8 scale factors
SH = 8.0
SU = 16.0
SA = 16.0
SW = 16.0
SVP = 16.0
SATT = 0.5
EXP_SCALE = 1.0 / (SU * SH)
EXP_BIAS = math.log(SATT)
U_SCALE = SU / (SA * SH)
VP_SCALE = SVP / (SW * SH)

TRACE = False
DEBUG = False
LAST_EXEC_NS = None
LAST_TRACE_PATH = None


def _build_nc(with_bias=False):
    import concourse.bass as bass
    import concourse.mybir as mybir
    import concourse.tile as tile
    from concourse import bacc

    f32 = mybir.dt.float32
    f32r = mybir.dt.float32r
    f8 = mybir.dt.float8e4
    DR = mybir.MatmulPerfMode.DoubleRowSwInterleave
    Ident = mybir.ActivationFunctionType.Identity
    Exp = mybir.ActivationFunctionType.Exp
    Sqrt = mybir.ActivationFunctionType.Sqrt
    Recip = mybir.ActivationFunctionType.Reciprocal
    mult = mybir.AluOpType.mult
    add = mybir.AluOpType.add
    subtract = mybir.AluOpType.subtract

    nc = bacc.Bacc()

    x_d = nc.declare_dram_parameter("x", [C, W], f32, isOutput=False)
    # fp8 weights packed as f32 words (4 fp8/word), bitcast on DMA:
    # layout [p, pair, k, col] flattened; see host packing in kernel().
    aq_d = nc.declare_dram_parameter("aq", [P, 512], f32, isOutput=False)
    wq_d = nc.declare_dram_parameter("wq8", [P, 512], f32, isOutput=False)
    # aux f32: [0:512] S selector, [512:1024] ST selector, [1024:1028] Su*g,
    # [1028:1032] Sh*gamma, [1032:1036] Sh*beta (CT cols each)
    aux_d = nc.declare_dram_parameter("aux", [P, 1036], f32, isOutput=False)
    # auxr f32 (consumed as f32r): [0:512] recbc lhsT (4 blocks; block g
    # has row 32g = Svp); with bias, [512:2560] bp lhsT (4 blocks of 512;
    # block g has row 32g = Svp*bp_eff)
    auxr_cols = 2560 if with_bias else 512
    auxr_d = nc.declare_dram_parameter("auxr", [P, auxr_cols], f32,
                                       isOutput=False)
    out_d = nc.declare_dram_parameter("out", [C, W], f32, isOutput=True)

    with tile.TileContext(nc) as tc:
        with (
            tc.tile_pool(name="singles", bufs=1) as singles,
            tc.tile_pool(name="xp", bufs=1) as xp,
            tc.tile_pool(name="hq", bufs=1) as hqp,
            tc.tile_pool(name="attp", bufs=1) as attp,
            tc.tile_pool(name="gn", bufs=2) as gnp,
            tc.tile_pool(name="outp", bufs=1) as outp,
        ):
            # ---- SBUF singles + input DMAs (tiny first) ----
            eps_t = singles.tile([P, 1], f32, name="eps_t")
            nc.vector.memset(eps_t, EPS)
            aux_sb = singles.tile([P, 1036], f32, name="aux_sb")
            nc.sync.dma_start(out=aux_sb, in_=aux_d[:, :])
            s_sb = aux_sb[:, 0:512].rearrange("p (t g) -> p t g", t=CT)
            st_sb = aux_sb[:, 512:1024].rearrange("p (t c) -> p t c", t=CT)
            g_sb = aux_sb[:, 1024:1028]
            gam_sb = aux_sb[:, 1028:1032]
            bet_sb = aux_sb[:, 1032:1036]
            auxr_sb = singles.tile([P, auxr_cols], f32r, name="auxr_sb")
            nc.sync.dma_start(out=auxr_sb, in_=auxr_d[:, :].bitcast(f32r))
            recbc_w = auxr_sb[:, 0:512]
            bp_w = auxr_sb[:, 512:auxr_cols] if with_bias else None

            # x in [16, 2048] row-slices: 8 slices/tile spread over the DMA
            # queues so a tile lands in ~6us instead of ~16us (a DMA issues
            # one descriptor per partition row, serially per queue)
            x_sb = [xp.tile([P, W], f32, name=f"x{t}") for t in range(CT)]
            for t in range(CT):
                for r in range(8):
                    nc.sync.dma_start(
                        out=x_sb[t][r * 16:(r + 1) * 16, :],
                        in_=x_d[t * P + r * 16:t * P + (r + 1) * 16, :])

            # A packed SwInterleaved on host: [p, pr, o-col, k]
            aq_sb = singles.tile([P, NPR, 512, 2], f8, name="aq_sb")
            nc.sync.dma_start(
                out=aq_sb.rearrange("p a b c -> p (a b c)"),
                in_=aq_d[:, :].bitcast(f8))
            wq_sb = singles.tile([P, NPR, 2, 512], f8, name="wq_sb")
            nc.sync.dma_start(
                out=wq_sb.rearrange("p a b c -> p (a b c)"),
                in_=wq_d[:, :].bitcast(f8))

            # fp8 ones [128, 2, 16] for the plain-DR row-sum reduction
            # (SwInterleave rejects 16-wide stationaries)
            onesf = singles.tile([P, 32], f32, name="onesf")
            nc.vector.memset(onesf, 1.0)
            ones8 = singles.tile([P, 2, 16], f8, name="ones8")
            nc.vector.tensor_copy(out=ones8.rearrange("p a b -> p (a b)"),
                                  in_=onesf)
            # f32r zeroed rhs tile; row 0 (rowsum) written per g
            zf = singles.tile([P, 512], f32, name="zf")
            nc.vector.memset(zf, 0.0)
            rzr = singles.tile([P, 512], f32r, name="rzr")
            nc.vector.tensor_copy(out=rzr, in_=zf)
            expb = singles.tile([P, 1], f32, name="expb")
            nc.vector.memset(expb, EXP_BIAS)

            h_q = [hqp.tile([P, NPR, W], f8, name=f"hq{p}") for p in range(2)]
            # SwInterleaved copies of h (lhsT for the sT/vp matmuls):
            # [p, col, k] so plane k is the stride-2 slice [:, :, k]
            hsw = [hqp.tile([P, W, NPR], f8, name=f"hsw{p}") for p in range(2)]
            u_q = [hqp.tile([P, NPR, W], f8, name=f"uq{p}") for p in range(2)]
            vp_q = [hqp.tile([P, 512, NPR], f8, name=f"vq{p}")
                    for p in range(IT // 2)]
            attT = [attp.tile([P, NPR, W], f8, name=f"at{p}")
                    for p in range(IT // 2)]

            # ================= GroupNorm -> h_q =================
            ps_gn_cm = tc.tile_pool(name="psgn", bufs=1, space="PSUM")
            ps_gn = ps_gn_cm.__enter__()

            def emit_gn_stats(t):
                stats = gnp.tile([P, NW, 6], f32, tag="bnstats", name=f"bns{t}")
                for sg in range(NW):
                    nc.vector.bn_stats(out=stats[:, sg, :],
                                       in_=x_sb[t][:, sg * 512:(sg + 1) * 512])
                mv = gnp.tile([P, 2], f32, tag="mv", name=f"mv{t}")
                nc.vector.bn_aggr(out=mv, in_=stats)
                st2 = gnp.tile([P, 2], f32, tag=f"st2_{t}", name=f"st2_{t}")
                nc.vector.tensor_copy(out=st2[:, 0:1], in_=mv[:, 0:1])
                nc.vector.tensor_tensor(out=st2[:, 1:2], in0=mv[:, 0:1],
                                        in1=mv[:, 0:1], op=mult)
                nc.vector.tensor_add(out=st2[:, 1:2], in0=st2[:, 1:2],
                                     in1=mv[:, 1:2])
                return st2

            def emit_gn_chain(t, st2):
                # group-sum then broadcast-back matmuls back to back (raw
                # moments broadcast; sqrt/reciprocal done per channel after)
                # so the chain has no PE<->DVE<->ACT ping-pong between mms.
                ps_g = ps_gn.tile([P, 2], f32, tag="gps", bufs=4,
                                  name=f"ps_g{t}")
                nc.tensor.matmul(ps_g[:], lhsT=s_sb[:, t, :], rhs=st2,
                                 start=True, stop=True)
                gsr = gnp.tile([P, 2], f32, tag="gsr", name=f"gsr{t}")
                nc.vector.memset(gsr, 0.0)
                nc.vector.tensor_copy(out=gsr[:8, :], in_=ps_g[:8, :])
                ps_bc = ps_gn.tile([P, 2], f32, tag="gps", bufs=4,
                                   name=f"psbc{t}")
                nc.tensor.matmul(ps_bc[:], lhsT=st_sb[:, t, :],
                                 rhs=gsr, start=True, stop=True)
                bca = gnp.tile([P, 2], f32, tag="bca", name=f"bca{t}")
                nc.vector.tensor_copy(out=bca, in_=ps_bc)
                sig = gnp.tile([P, 1], f32, tag="sig", name=f"sig{t}")
                nc.vector.tensor_tensor(out=sig, in0=bca[:, 0:1],
                                        in1=bca[:, 0:1], op=mult)
                nc.vector.tensor_tensor(out=sig, in0=bca[:, 1:2],
                                        in1=sig, op=subtract)
                nc.scalar.activation(out=sig, in_=sig,
                                     func=Sqrt, bias=eps_t, scale=1.0)
                nc.vector.reciprocal(sig, sig)
                alph = gnp.tile([P, 1], f32, tag=f"alph{t}", name=f"alph{t}")
                nc.vector.tensor_tensor(out=alph, in0=sig,
                                        in1=gam_sb[:, t:t + 1], op=mult)
                beta = gnp.tile([P, 1], f32, tag=f"beta{t}", name=f"beta{t}")
                nc.vector.tensor_tensor(out=beta, in0=bca[:, 0:1],
                                        in1=alph, op=mult)
                nc.vector.tensor_tensor(out=beta, in0=bet_sb[:, t:t + 1],
                                        in1=beta, op=subtract)
                if t < 2:
                    # front tiles on ACT (DVE is busy with the t2/t3 stats)
                    nc.scalar.activation(out=h_q[t // 2][:, t % 2, :],
                                         in_=x_sb[t],
                                         func=Ident, scale=alph, bias=beta)
                else:
                    nc.vector.tensor_scalar(out=h_q[t // 2][:, t % 2, :],
                                            in0=x_sb[t],
                                            scalar1=alph, scalar2=beta,
                                            op0=mult, op1=add)

            # pair 0 fully first so the u/vp pr=0 matmuls can start early
            st2_0 = emit_gn_stats(0)
            st2_1 = emit_gn_stats(1)
            emit_gn_chain(0, st2_0)
            emit_gn_chain(1, st2_1)
            # interleaved copies for the vp/sT lhsT on DVE (~2.9us each;
            # GpSimd takes ~14us and stalls the vp matmuls)
            st2_2 = emit_gn_stats(2)
            nc.vector.tensor_copy(out=hsw[0].rearrange("p w k -> p k w"),
                                  in_=h_q[0])
            st2_3 = emit_gn_stats(3)
            emit_gn_chain(2, st2_2)
            emit_gn_chain(3, st2_3)
            nc.vector.tensor_copy(out=hsw[1].rearrange("p w k -> p k w"),
                                  in_=h_q[1])
            ps_gn_cm.__exit__(None, None, None)

            # ================= u_q and vp_q (fp8 DoubleRow) =========
            ps_uv_cm = tc.tile_pool(name="psuv", bufs=1, space="PSUM")
            ps_uv = ps_uv_cm.__enter__()
            for ot in range(CT):
                ups = ps_uv.tile([P, W], f32, tag="ubig", bufs=1,
                                 name=f"ups{ot}")
                for jc in range(NW):
                    for pr in range(NPR):
                        nc.tensor.matmul(
                            ups[:, jc * 512:(jc + 1) * 512],
                            lhsT=aq_sb[:, pr, ot * P:(ot + 1) * P, :],
                            rhs=h_q[pr][:, :, jc * 512:(jc + 1) * 512],
                            start=(pr == 0), stop=(pr == NPR - 1),
                            perf_mode=DR)
                nc.scalar.activation(out=u_q[ot // 2][:, ot % 2, :], in_=ups,
                                     func=Ident, scale=U_SCALE,
                                     bias=g_sb[:, ot:ot + 1])
                for jt in range(4 * ot, 4 * ot + 4):
                    vps = ps_uv.tile([P, 512], f32, tag="v512", bufs=4,
                                     name=f"vps{jt}")
                    for pr in range(NPR):
                        nc.tensor.matmul(
                            vps[:],
                            lhsT=hsw[pr][:, jt * P:(jt + 1) * P, :],
                            rhs=wq_sb[:, pr, :, :],
                            start=(pr == 0), stop=(pr == NPR - 1),
                            perf_mode=DR)
                    nc.vector.tensor_scalar_mul(vp_q[jt // 2][:, :, jt % 2],
                                                vps, VP_SCALE)
            ps_uv_cm.__exit__(None, None, None)

            # ================= sT -> exp -> attT (fp8) =========
            ps_st_cm = tc.tile_pool(name="psst", bufs=1, space="PSUM")
            ps_st = ps_st_cm.__enter__()
            for jt in range(IT):
                for half in range(2):
                    sps = ps_st.tile([P, W // 2], f32, tag="stbig", bufs=3,
                                     name=f"sps{jt}_{half}")
                    for gh in range(NW // 2):
                        g = half * 2 + gh
                        for pr in range(NPR):
                            nc.tensor.matmul(
                                sps[:, gh * 512:(gh + 1) * 512],
                                lhsT=hsw[pr][:, jt * P:(jt + 1) * P, :],
                                rhs=u_q[pr][:, :, g * 512:(g + 1) * 512],
                                start=(pr == 0), stop=(pr == NPR - 1),
                                perf_mode=DR)
                    nc.scalar.activation(
                        out=attT[jt // 2][:, jt % 2,
                                          half * 1024:(half + 1) * 1024],
                        in_=sps, func=Exp, scale=EXP_SCALE, bias=expb)
            ps_st_cm.__exit__(None, None, None)

            # ============ row sums, normalize, ho, residual =========
            ps_g_cm = tc.tile_pool(name="psg", bufs=1, space="PSUM")
            ps_gl = ps_g_cm.__enter__()
            # All 4 groups' row sums + reciprocals first: the ~4us DVE
            # reciprocals then overlap the ho matmul stream instead of
            # gating each group's output stage.
            recbc_l = []
            for g in range(NW):
                prow = ps_gl.tile([16, 512], f32, tag="prow", bufs=1,
                                  name=f"prow{g}")
                for pp in range(IT // 2):
                    nc.tensor.matmul(
                        prow[:],
                        lhsT=ones8[:, :, :],
                        rhs=attT[pp][:, :, g * 512:(g + 1) * 512],
                        start=(pp == 0), stop=(pp == IT // 2 - 1),
                        perf_mode=mybir.MatmulPerfMode.DoubleRow)
                nc.vector.tensor_copy(out=rzr[32 * g:32 * g + 1, :],
                                      in_=prow[0:1, :])
                prb = ps_gl.tile([P, 512], f32, tag="prb", bufs=3,
                                 name=f"prb{g}")
                nc.tensor.matmul(prb[:],
                                 lhsT=recbc_w[:, g * P:(g + 1) * P],
                                 rhs=rzr, start=True, stop=True)
                recbc_l.append(prb)
            for g in range(NW):
                # reciprocal emitted here so the DVE queue interleaves it
                # with the previous group's output stage (not 4 in a row
                # ahead of every multiply)
                recbc = outp.tile([P, 512], f32, tag="recbc", bufs=2,
                                  name=f"recbc{g}")
                nc.vector.reciprocal(recbc, recbc_l[g])
                for ot in range(CT):
                    pho = ps_gl.tile([P, 512], f32, tag="pho", bufs=4,
                                     name=f"pho{g}_{ot}")
                    if with_bias:
                        nc.tensor.matmul(
                            pho[:],
                            lhsT=bp_w[:, g * 512 + ot * P:
                                      g * 512 + (ot + 1) * P],
                            rhs=rzr, start=True, stop=False,
                            skip_group_check=True)
                    for pp in range(IT // 2):
                        nc.tensor.matmul(
                            pho[:],
                            lhsT=vp_q[pp][:, ot * P:(ot + 1) * P, :],
                            rhs=attT[pp][:, :, g * 512:(g + 1) * 512],
                            start=(not with_bias and pp == 0),
                            stop=(pp == IT // 2 - 1),
                            perf_mode=DR, skip_group_check=True)
                    tmp = outp.tile([P, 512], f32, tag="tmp", bufs=4,
                                    name=f"tmp{g}_{ot}")
                    nc.vector.tensor_tensor(out=tmp, in0=pho, in1=recbc,
                                            op=mult)
                    osb = outp.tile([P, 512], f32, tag="osb", bufs=4,
                                    name=f"osb{g}_{ot}")
                    nc.vector.tensor_tensor(
                        out=osb, in0=tmp,
                        in1=x_sb[ot][:, g * 512:(g + 1) * 512], op=add)
                    if g == NW - 1:
                        # last wave: quarter the store so the kernel tail is
                        # ~3.5us instead of one 14us single-queue drain
                        for r in range(4):
                            nc.sync.dma_start(
                                out=out_d[ot * P + r * 32:
                                          ot * P + (r + 1) * 32,
                                          g * 512:(g + 1) * 512],
                                in_=osb[r * 32:(r + 1) * 32, :])
                    else:
                        nc.sync.dma_start(
                            out=out_d[ot * P:(ot + 1) * P,
                                      g * 512:(g + 1) * 512],
                            in_=osb)
            ps_g_cm.__exit__(None, None, None)

    nc.finalize()
    return nc


@functools.lru_cache(maxsize=2)
def _built(with_bias=False):
    return _build_nc(with_bias)


def _pack_fp8(a):
    """[128, 2048] float -> e4m3 bytes -> [128, 512] f32 view."""
    import ml_dtypes
    q = np.ascontiguousarray(a.astype(np.float32)).astype(ml_dtypes.float8_e4m3)
    return np.ascontiguousarray(q).view(np.uint8).view(np.float32)


def kernel(x, gn_gamma, gn_beta, wq, bq, wk, bk, wv, bv, wp, bp):
    global LAST_EXEC_NS, LAST_TRACE_PATH
    from concourse.bass_utils import run_bass_kernel_spmd

    x = np.asarray(x, dtype=np.float32)
    f = np.float32
    f64 = np.float64
    scale = float(C) ** -0.5
    wq64 = np.asarray(wq, f64)
    wk64 = np.asarray(wk, f64)
    wv64 = np.asarray(wv, f64)
    wp64 = np.asarray(wp, f64)

    A = (wq64.T @ wk64 * scale)          # (c_in, o): scores = h.T A h + g.h
    Wv = (wp64 @ wv64)                   # (c_out, c_in): vp = Wv h
    g_vec = (wk64.T @ (np.asarray(bq, f64) * scale)).astype(f)
    bp_eff = (np.asarray(bp, f64) + wp64 @ np.asarray(bv, f64)).astype(f)

    # A: SwInterleaved weights [p, pair, col, k] with columns reversed per
    # 128-block (the hw reads interleaved pairs in reverse column order, so
    # this yields naturally-ordered output partitions).
    Ar = (SA * A).astype(f).reshape(2, 2, P, 4, P)[..., ::-1]
    aq = _pack_fp8(Ar.transpose(2, 0, 3, 4, 1).reshape(P, 2048))
    # W stays plane-major (it is the moving operand) but its free columns
    # are pre-reversed per 128-block to cancel the reversal that the
    # SwInterleave ho lhsT (vp) introduces in the output partitions.
    Wr = (SW * Wv.T).astype(f).reshape(2, 2, P, 4, P)[..., ::-1]
    wq8 = _pack_fp8(Wr.reshape(2, 2, P, 512).transpose(2, 0, 1, 3)
                    .reshape(P, 2048))

    gsz = C // G
    aux = np.zeros((P, 1036), dtype=f)
    idx = np.arange(P)
    for t in range(CT):
        aux[idx, t * P + idx // gsz] = 1.0 / gsz                # S selector
        aux[idx // gsz, 512 + t * P + idx] = 1.0                # ST selector
    aux[:, 1024:1028] = (SU * g_vec).reshape(CT, P).T
    aux[:, 1028:1032] = (SH * np.asarray(gn_gamma, f)).reshape(CT, P).T
    aux[:, 1032:1036] = (SH * np.asarray(gn_beta, f)).reshape(CT, P).T

    # recbc matmul broadcasts Svp*Satt*rowsum; reciprocal then yields the
    # final per-column normalizer 1/(Svp*Satt*sum(exp)).
    with_bias = bool(np.any(bp_eff != 0.0))
    auxr = np.zeros((P, 2560 if with_bias else 512), dtype=f)
    for g in range(NW):
        auxr[32 * g, g * P:(g + 1) * P] = SVP
        if with_bias:
            auxr[32 * g, 512 + g * 512:512 + (g + 1) * 512] = SVP * bp_eff

    shared = dict(aq=aq, wq8=wq8, aux=aux, auxr=auxr)
    in_maps = [dict(x=np.ascontiguousarray(x[i]), **shared) for i in range(B)]

    nc = _built(with_bias)
    last_err = None
    for attempt in range(3):
        try:
            res = run_bass_kernel_spmd(nc, in_maps, list(range(B)), trace=TRACE)
            out = np.stack([np.asarray(res.results[i]["out"], dtype=np.float32)
                            for i in range(B)], axis=0)
            break
        except Exception as e:  # transient NRT device errors: retry
            last_err = e
            if attempt == 2:
                raise
            import time
            time.sleep(2.0)
    if TRACE:
        LAST_EXEC_NS = res.exec_time_ns
        if res.instructions_and_trace is not None:
            LAST_TRACE_PATH = res.instructions_and_trace[1]
    return out


# revision 68
# speedup vs baseline: 1.1263x; 1.1263x over previous
"""AttnBlock (GroupNorm -> QKV -> full attention -> proj + residual) on 8
Trainium2 NeuronCores, data-parallel over batch (b=8, one sample per core).

Transposeless fp8 design. Per core (sample):
  h_q = fp8(Sh*GroupNorm(x)); A = (wq.T wk)/sqrt(c), W = wp wv folded on
  host to fp8 (k / final proj never computed).
  u_q = fp8(Su*(A.T h + g)) via fp8 DoubleRow matmuls (K=256/pass).
  sT[j,i] = scores transposed, computed directly (lhsT = h j-cols), so no
  PE transposes and no PSUM->SBUF att copies are needed; exp written
  straight to fp8 attT slabs by ACT (scale/bias folded into the exp).
  Row sums of the quantized exp via a fp8 DoubleRow ones-matmul; the
  softmax normalization is applied per-column at the output stage using a
  PE-broadcast reciprocal row. ho = vp.T @ attT in fp8 DoubleRow; bias via
  a 1-row f32r matmul into the same PSUM accumulation; residual added on
  DVE from resident x tiles.
"""

import functools
import math

import numpy as np

B = 8
C = 512
W = 2048
G = 32
EPS = 1e-6
P = 128
CT = C // P          # 4 channel tiles
NW = W // 512        # 4 w-chunks of 512
IT = W // P          # 16 j-tiles
NPR = 2              # contraction pairs (2 x 256 = 512)

# fp8 scale factors
SH = 8.0
SU = 16.0
SA = 16.0
SW = 16.0
SVP = 16.0
SATT = 0.5
EXP_SCALE = 1.0 / (SU * SH)
EXP_BIAS = math.log(SATT)
U_SCALE = SU / (SA * SH)
VP_SCALE = SVP / (SW * SH)

TRACE = False
DEBUG = False
LAST_EXEC_NS = None
LAST_TRACE_PATH = None


def _build_nc(with_bias=False):
    import concourse.bass as bass
    import concourse.mybir as mybir
    import concourse.tile as tile
    from concourse import bacc

    f32 = mybir.dt.float32
    f32r = mybir.dt.float32r
    f8 = mybir.dt.float8e4
    DR = mybir.MatmulPerfMode.DoubleRow
    Ident = mybir.ActivationFunctionType.Identity
    Exp = mybir.ActivationFunctionType.Exp
    Sqrt = mybir.ActivationFunctionType.Sqrt
    Recip = mybir.ActivationFunctionType.Reciprocal
    mult = mybir.AluOpType.mult
    add = mybir.AluOpType.add
    subtract = mybir.AluOpType.subtract

    nc = bacc.Bacc()

    x_d = nc.declare_dram_parameter("x", [C, W], f32, isOutput=False)
    # fp8 weights packed as f32 words (4 fp8/word), bitcast on DMA:
    # layout [p, pair, k, col] flattened; see host packing in kernel().
    aq_d = nc.declare_dram_parameter("aq", [P, 512], f32, isOutput=False)
    wq_d = nc.declare_dram_parameter("wq8", [P, 512], f32, isOutput=False)
    # aux f32: [0:512] S selector, [512:1024] ST selector, [1024:1028] Su*g,
    # [1028:1032] Sh*gamma, [1032:1036] Sh*beta (CT cols each)
    aux_d = nc.declare_dram_parameter("aux", [P, 1036], f32, isOutput=False)
    # auxr f32 (consumed as f32r): [0:512] recbc lhsT (4 blocks; block g
    # has row 32g = Svp); with bias, [512:2560] bp lhsT (4 blocks of 512;
    # block g has row 32g = Svp*bp_eff)
    auxr_cols = 2560 if with_bias else 512
    auxr_d = nc.declare_dram_parameter("auxr", [P, auxr_cols], f32,
                                       isOutput=False)
    out_d = nc.declare_dram_parameter("out", [C, W], f32, isOutput=True)

    with tile.TileContext(nc) as tc:
        with (
            tc.tile_pool(name="singles", bufs=1) as singles,
            tc.tile_pool(name="xp", bufs=1) as xp,
            tc.tile_pool(name="hq", bufs=1) as hqp,
            tc.tile_pool(name="attp", bufs=1) as attp,
            tc.tile_pool(name="gn", bufs=2) as gnp,
            tc.tile_pool(name="outp", bufs=1) as outp,
        ):
            # ---- SBUF singles + input DMAs (tiny first) ----
            eps_t = singles.tile([P, 1], f32, name="eps_t")
            nc.vector.memset(eps_t, EPS)
            aux_sb = singles.tile([P, 1036], f32, name="aux_sb")
            nc.sync.dma_start(out=aux_sb, in_=aux_d[:, :])
            s_sb = aux_sb[:, 0:512].rearrange("p (t g) -> p t g", t=CT)
            st_sb = aux_sb[:, 512:1024].rearrange("p (t c) -> p t c", t=CT)
            g_sb = aux_sb[:, 1024:1028]
            gam_sb = aux_sb[:, 1028:1032]
            bet_sb = aux_sb[:, 1032:1036]
            auxr_sb = singles.tile([P, auxr_cols], f32r, name="auxr_sb")
            nc.sync.dma_start(out=auxr_sb, in_=auxr_d[:, :].bitcast(f32r))
            recbc_w = auxr_sb[:, 0:512]
            bp_w = auxr_sb[:, 512:auxr_cols] if with_bias else None

            x_sb = [xp.tile([P, W], f32, name=f"x{t}") for t in range(CT)]
            for t in range(CT):
                for hw in range(2):
                    nc.sync.dma_start(
                        out=x_sb[t][:, hw * 1024:(hw + 1) * 1024],
                        in_=x_d[t * P:(t + 1) * P, hw * 1024:(hw + 1) * 1024])

            aq_sb = singles.tile([P, NPR, 2, 512], f8, name="aq_sb")
            nc.sync.dma_start(
                out=aq_sb.rearrange("p a b c -> p (a b c)"),
                in_=aq_d[:, :].bitcast(f8))
            wq_sb = singles.tile([P, NPR, 2, 512], f8, name="wq_sb")
            nc.sync.dma_start(
                out=wq_sb.rearrange("p a b c -> p (a b c)"),
                in_=wq_d[:, :].bitcast(f8))

            # fp8 ones [128, 2, 16] for the plain-DR row-sum reduction
            # (SwInterleave rejects 16-wide stationaries)
            onesf = singles.tile([P, 32], f32, name="onesf")
            nc.vector.memset(onesf, 1.0)
            ones8 = singles.tile([P, 2, 16], f8, name="ones8")
            nc.vector.tensor_copy(out=ones8.rearrange("p a b -> p (a b)"),
                                  in_=onesf)
            # f32r zeroed rhs tile; row 0 (rowsum) written per g
            zf = singles.tile([P, 512], f32, name="zf")
            nc.vector.memset(zf, 0.0)
            rzr = singles.tile([P, 512], f32r, name="rzr")
            nc.vector.tensor_copy(out=rzr, in_=zf)
            expb = singles.tile([P, 1], f32, name="expb")
            nc.vector.memset(expb, EXP_BIAS)

            h_q = [hqp.tile([P, NPR, W], f8, name=f"hq{p}") for p in range(2)]
            u_q = [hqp.tile([P, NPR, W], f8, name=f"uq{p}") for p in range(2)]
            vp_q = [hqp.tile([P, NPR, 512], f8, name=f"vq{p}")
                    for p in range(IT // 2)]
            attT = [attp.tile([P, NPR, W], f8, name=f"at{p}")
                    for p in range(IT // 2)]

            # ================= GroupNorm -> h_q =================
            ps_gn_cm = tc.tile_pool(name="psgn", bufs=1, space="PSUM")
            ps_gn = ps_gn_cm.__enter__()

            def emit_gn_stats(t):
                stats = gnp.tile([P, NW, 6], f32, tag="bnstats", name=f"bns{t}")
                for sg in range(NW):
                    nc.vector.bn_stats(out=stats[:, sg, :],
                                       in_=x_sb[t][:, sg * 512:(sg + 1) * 512])
                mv = gnp.tile([P, 2], f32, tag="mv", name=f"mv{t}")
                nc.vector.bn_aggr(out=mv, in_=stats)
                st2 = gnp.tile([P, 2], f32, tag=f"st2_{t}", name=f"st2_{t}")
                nc.vector.tensor_copy(out=st2[:, 0:1], in_=mv[:, 0:1])
                nc.vector.tensor_tensor(out=st2[:, 1:2], in0=mv[:, 0:1],
                                        in1=mv[:, 0:1], op=mult)
                nc.vector.tensor_add(out=st2[:, 1:2], in0=st2[:, 1:2],
                                     in1=mv[:, 1:2])
                return st2

            def emit_gn_chain(t, st2):
                # group-sum then broadcast-back matmuls back to back (raw
                # moments broadcast; sqrt/reciprocal done per channel after)
                # so the chain has no PE<->DVE<->ACT ping-pong between mms.
                ps_g = ps_gn.tile([P, 2], f32, tag="gps", bufs=4,
                                  name=f"ps_g{t}")
                nc.tensor.matmul(ps_g[:], lhsT=s_sb[:, t, :], rhs=st2,
                                 start=True, stop=True)
                gsr = gnp.tile([P, 2], f32, tag="gsr", name=f"gsr{t}")
                nc.vector.memset(gsr, 0.0)
                nc.vector.tensor_copy(out=gsr[:8, :], in_=ps_g[:8, :])
                ps_bc = ps_gn.tile([P, 2], f32, tag="gps", bufs=4,
                                   name=f"psbc{t}")
                nc.tensor.matmul(ps_bc[:], lhsT=st_sb[:, t, :],
                                 rhs=gsr, start=True, stop=True)
                bca = gnp.tile([P, 2], f32, tag="bca", name=f"bca{t}")
                nc.vector.tensor_copy(out=bca, in_=ps_bc)
                sig = gnp.tile([P, 1], f32, tag="sig", name=f"sig{t}")
                nc.vector.tensor_tensor(out=sig, in0=bca[:, 0:1],
                                        in1=bca[:, 0:1], op=mult)
                nc.vector.tensor_tensor(out=sig, in0=bca[:, 1:2],
                                        in1=sig, op=subtract)
                nc.scalar.activation(out=sig, in_=sig,
                                     func=Sqrt, bias=eps_t, scale=1.0)
                nc.vector.reciprocal(sig, sig)
                alph = gnp.tile([P, 1], f32, tag=f"alph{t}", name=f"alph{t}")
                nc.vector.tensor_tensor(out=alph, in0=sig,
                                        in1=gam_sb[:, t:t + 1], op=mult)
                beta = gnp.tile([P, 1], f32, tag=f"beta{t}", name=f"beta{t}")
                nc.vector.tensor_tensor(out=beta, in0=bca[:, 0:1],
                                        in1=alph, op=mult)
                nc.vector.tensor_tensor(out=beta, in0=bet_sb[:, t:t + 1],
                                        in1=beta, op=subtract)
                if t < 2:
                    # front tiles on ACT (DVE is busy with the t2/t3 stats)
                    nc.scalar.activation(out=h_q[t // 2][:, t % 2, :],
                                         in_=x_sb[t],
                                         func=Ident, scale=alph, bias=beta)
                else:
                    nc.vector.tensor_scalar(out=h_q[t // 2][:, t % 2, :],
                                            in0=x_sb[t],
                                            scalar1=alph, scalar2=beta,
                                            op0=mult, op1=add)

            # pair 0 fully first so the u/vp pr=0 matmuls can start early
            st2_0 = emit_gn_stats(0)
            st2_1 = emit_gn_stats(1)
            emit_gn_chain(0, st2_0)
            emit_gn_chain(1, st2_1)
            st2_2 = emit_gn_stats(2)
            st2_3 = emit_gn_stats(3)
            emit_gn_chain(2, st2_2)
            emit_gn_chain(3, st2_3)
            ps_gn_cm.__exit__(None, None, None)

            # ================= u_q and vp_q (fp8 DoubleRow) =========
            ps_uv_cm = tc.tile_pool(name="psuv", bufs=1, space="PSUM")
            ps_uv = ps_uv_cm.__enter__()
            for ot in range(CT):
                ups = ps_uv.tile([P, W], f32, tag="ubig", bufs=1,
                                 name=f"ups{ot}")
                for jc in range(NW):
                    for pr in range(NPR):
                        nc.tensor.matmul(
                            ups[:, jc * 512:(jc + 1) * 512],
                            lhsT=aq_sb[:, pr, :, ot * P:(ot + 1) * P],
                            rhs=h_q[pr][:, :, jc * 512:(jc + 1) * 512],
                            start=(pr == 0), stop=(pr == NPR - 1),
                            perf_mode=DR)
                nc.scalar.activation(out=u_q[ot // 2][:, ot % 2, :], in_=ups,
                                     func=Ident, scale=U_SCALE,
                                     bias=g_sb[:, ot:ot + 1])
                for jt in range(4 * ot, 4 * ot + 4):
                    vps = ps_uv.tile([P, 512], f32, tag="v512", bufs=4,
                                     name=f"vps{jt}")
                    for pr in range(NPR):
                        nc.tensor.matmul(
                            vps[:],
                            lhsT=h_q[pr][:, :, jt * P:(jt + 1) * P],
                            rhs=wq_sb[:, pr, :, :],
                            start=(pr == 0), stop=(pr == NPR - 1),
                            perf_mode=DR)
                    nc.vector.tensor_scalar_mul(vp_q[jt // 2][:, jt % 2, :],
                                                vps, VP_SCALE)
            ps_uv_cm.__exit__(None, None, None)

            # ================= sT -> exp -> attT (fp8) =========
            ps_st_cm = tc.tile_pool(name="psst", bufs=1, space="PSUM")
            ps_st = ps_st_cm.__enter__()
            for jt in range(IT):
                for half in range(2):
                    sps = ps_st.tile([P, W // 2], f32, tag="stbig", bufs=3,
                                     name=f"sps{jt}_{half}")
                    for gh in range(NW // 2):
                        g = half * 2 + gh
                        for pr in range(NPR):
                            nc.tensor.matmul(
                                sps[:, gh * 512:(gh + 1) * 512],
                                lhsT=h_q[pr][:, :, jt * P:(jt + 1) * P],
                                rhs=u_q[pr][:, :, g * 512:(g + 1) * 512],
                                start=(pr == 0), stop=(pr == NPR - 1),
                                perf_mode=DR)
                    nc.scalar.activation(
                        out=attT[jt // 2][:, jt % 2,
                                          half * 1024:(half + 1) * 1024],
                        in_=sps, func=Exp, scale=EXP_SCALE, bias=expb)
            ps_st_cm.__exit__(None, None, None)

            # ============ row sums, normalize, ho, residual =========
            ps_g_cm = tc.tile_pool(name="psg", bufs=1, space="PSUM")
            ps_gl = ps_g_cm.__enter__()
            # All 4 groups' row sums + reciprocals first: the ~4us DVE
            # reciprocals then overlap the ho matmul stream instead of
            # gating each group's output stage.
            recbc_l = []
            for g in range(NW):
                prow = ps_gl.tile([16, 512], f32, tag="prow", bufs=1,
                                  name=f"prow{g}")
                for pp in range(IT // 2):
                    nc.tensor.matmul(
                        prow[:],
                        lhsT=ones8[:, :, :],
                        rhs=attT[pp][:, :, g * 512:(g + 1) * 512],
                        start=(pp == 0), stop=(pp == IT // 2 - 1),
                        perf_mode=DR)
                nc.vector.tensor_copy(out=rzr[32 * g:32 * g + 1, :],
                                      in_=prow[0:1, :])
                prb = ps_gl.tile([P, 512], f32, tag="prb", bufs=3,
                                 name=f"prb{g}")
                nc.tensor.matmul(prb[:],
                                 lhsT=recbc_w[:, g * P:(g + 1) * P],
                                 rhs=rzr, start=True, stop=True)
                recbc_l.append(prb)
            for g in range(NW):
                # reciprocal emitted here so the DVE queue interleaves it
                # with the previous group's output stage (not 4 in a row
                # ahead of every multiply)
                recbc = outp.tile([P, 512], f32, tag="recbc", bufs=2,
                                  name=f"recbc{g}")
                nc.vector.reciprocal(recbc, recbc_l[g])
                for ot in range(CT):
                    pho = ps_gl.tile([P, 512], f32, tag="pho", bufs=4,
                                     name=f"pho{g}_{ot}")
                    if with_bias:
                        nc.tensor.matmul(
                            pho[:],
                            lhsT=bp_w[:, g * 512 + ot * P:
                                      g * 512 + (ot + 1) * P],
                            rhs=rzr, start=True, stop=False,
                            skip_group_check=True)
                    for pp in range(IT // 2):
                        nc.tensor.matmul(
                            pho[:],
                            lhsT=vp_q[pp][:, :, ot * P:(ot + 1) * P],
                            rhs=attT[pp][:, :, g * 512:(g + 1) * 512],
                            start=(not with_bias and pp == 0),
                            stop=(pp == IT // 2 - 1),
                            perf_mode=DR, skip_group_check=True)
                    tmp = outp.tile([P, 512], f32, tag="tmp", bufs=4,
                                    name=f"tmp{g}_{ot}")
                    nc.vector.tensor_tensor(out=tmp, in0=pho, in1=recbc,
                                            op=mult)
                    osb = outp.tile([P, 512], f32, tag="osb", bufs=4,
                                    name=f"osb{g}_{ot}")
                    nc.vector.tensor_tensor(
                        out=osb, in0=tmp,
                        in1=x_sb[ot][:, g * 512:(g + 1) * 512], op=add)
                    if g == NW - 1:
                        # last wave: quarter the store so the kernel tail is
                        # ~3.5us instead of one 14us single-queue drain
                        for r in range(4):
                            nc.sync.dma_start(
                                out=out_d[ot * P + r * 32:
                                          ot * P + (r + 1) * 32,
                                          g * 512:(g + 1) * 512],
                                in_=osb[r * 32:(r + 1) * 32, :])
                    else:
                        nc.sync.dma_start(
                            out=out_d[ot * P:(ot + 1) * P,
                                      g * 512:(g + 1) * 512],
                            in_=osb)
            ps_g_cm.__exit__(None, None, None)

    nc.finalize()
    return nc


@functools.lru_cache(maxsize=2)
def _built(with_bias=False):
    return _build_nc(with_bias)


def _pack_fp8(a):
    """[128, 2048] float -> e4m3 bytes -> [128, 512] f32 view."""
    import ml_dtypes
    q = np.ascontiguousarray(a.astype(np.float32)).astype(ml_dtypes.float8_e4m3)
    return np.ascontiguousarray(q).view(np.uint8).view(np.float32)


def kernel(x, gn_gamma, gn_beta, wq, bq, wk, bk, wv, bv, wp, bp):
    global LAST_EXEC_NS, LAST_TRACE_PATH
    from concourse.bass_utils import run_bass_kernel_spmd

    x = np.asarray(x, dtype=np.float32)
    f = np.float32
    f64 = np.float64
    scale = float(C) ** -0.5
    wq64 = np.asarray(wq, f64)
    wk64 = np.asarray(wk, f64)
    wv64 = np.asarray(wv, f64)
    wp64 = np.asarray(wp, f64)

    A = (wq64.T @ wk64 * scale)          # (c_in, o): scores = h.T A h + g.h
    Wv = (wp64 @ wv64)                   # (c_out, c_in): vp = Wv h
    g_vec = (wk64.T @ (np.asarray(bq, f64) * scale)).astype(f)
    bp_eff = (np.asarray(bp, f64) + wp64 @ np.asarray(bv, f64)).astype(f)

    def pack_pairs(m):
        # m: (c_in=512, col=512) -> [p, pair, k, col] -> packed f32 [128, 512]
        arr = m.reshape(2, 2, P, 512).transpose(2, 0, 1, 3).reshape(P, 2048)
        return _pack_fp8(arr)

    aq = pack_pairs((SA * A).astype(f))
    wq8 = pack_pairs((SW * Wv.T).astype(f))

    gsz = C // G
    aux = np.zeros((P, 1036), dtype=f)
    idx = np.arange(P)
    for t in range(CT):
        aux[idx, t * P + idx // gsz] = 1.0 / gsz                # S selector
        aux[idx // gsz, 512 + t * P + idx] = 1.0                # ST selector
    aux[:, 1024:1028] = (SU * g_vec).reshape(CT, P).T
    aux[:, 1028:1032] = (SH * np.asarray(gn_gamma, f)).reshape(CT, P).T
    aux[:, 1032:1036] = (SH * np.asarray(gn_beta, f)).reshape(CT, P).T

    # recbc matmul broadcasts Svp*Satt*rowsum; reciprocal then yields the
    # final per-column normalizer 1/(Svp*Satt*sum(exp)).
    with_bias = bool(np.any(bp_eff != 0.0))
    auxr = np.zeros((P, 2560 if with_bias else 512), dtype=f)
    for g in range(NW):
        auxr[32 * g, g * P:(g + 1) * P] = SVP
        if with_bias:
            auxr[32 * g, 512 + g * 512:512 + (g + 1) * 512] = SVP * bp_eff

    shared = dict(aq=aq, wq8=wq8, aux=aux, auxr=auxr)
    in_maps = [dict(x=np.ascontiguousarray(x[i]), **shared) for i in range(B)]

    nc = _built(with_bias)
    last_err = None
    for attempt in range(3):
        try:
            res = run_bass_kernel_spmd(nc, in_maps, list(range(B)), trace=TRACE)
            out = np.stack([np.asarray(res.results[i]["out"], dtype=np.float32)
                            for i in range(B)], axis=0)
            break
        except Exception as e:  # transient NRT device errors: retry
            last_err = e
            if attempt == 2:
                raise
            import time
            time.sleep(2.0)
    if TRACE:
        LAST_EXEC_NS = res.exec_time_ns
        if res.instructions_and_trace is not None:
            LAST_TRACE_PATH = res.instructions_and_trace[1]
    return out
